# revision 4
# baseline (speedup 1.0000x reference)
"""Lorentz multi-head attention on 8 Trainium2 NeuronCores.

Sharding: head-parallel phase 1 (core c computes head c for all batches:
QKV Lorentz projections, Lorentz-inner-product scores, softmax-free
exp-attention, Lorentz-midpoint normalize), then a PER-BATCH AllToAll
exchanges (head-block -> token-block) so phase 2 (concat_logradius fusion
+ output LorentzFC) runs token-parallel and overlaps later batches'
compute (core c handles tokens [c*256,(c+1)*256) of each batch).

Softmax denominator is skipped entirely: the Lorentz midpoint renormalizes
m / sqrt(K*(t^2-||s||^2)), which is invariant to positive row scaling, so
exp(scores) can be used unnormalized (scores are O(+-5), no overflow risk).

Biases are folded into the matmuls by augmenting tokens with a constant-1
column and weights with a bias row. sqrt/rsqrt are computed as
exp(+-0.5*ln(x)) so the ScalarEngine needs only the one
natural_log_exp_and_others table set (no ~2.7us table swaps).

Inputs arrive pre-transposed AND pre-cast to bf16 on the host (halves the
x DMA and removes all on-chip f32->bf16 casts of x / weights).

Score exp() runs on [128,1024] two-bank PSUM tiles (one ACTIVATE per
128-token key chunk x half the queries) to amortize the ~350-cycle
ACT pipeline fill; q and k time-rows come from one block-diagonal
ones-matmul + Ln-from-PSUM + a single [2,2048] Exp.
"""

import sys

sys.path.insert(0, "/opt/trn_rl_repo")

import numpy as np
import ml_dtypes

import concourse.bass as bass
import concourse.mybir as mybir
import concourse.tile as tile
from concourse import bacc, bass_utils
from concourse.masks import make_identity

# Problem constants (hardcoded per task contract)
B, N, D = 4, 2048, 513
H, DHS = 8, 64
NCORES = 8
KCURV = 0.1
INVK = 10.0
SCALE = 1.0 / np.sqrt(DHS)  # 0.125
S_CONST = 2.8479428291320801  # exp(0.5*(digamma(256)-digamma(32)))
DPAD = 640  # 513 padded to 5*128 (col 513 = constant-1 bias lane)
KC = 5  # contraction chunks of 128
BN = B * N  # 8192 tokens
TPC = N // NCORES  # 256 tokens per core per batch in phase 2
F32 = mybir.dt.float32
BF16 = mybir.dt.bfloat16
Ln = mybir.ActivationFunctionType.Ln
Exp = mybir.ActivationFunctionType.Exp

_CACHE = {}


def _patch_act_tables(nc):
    # Exp and Ln both live in the natural_log_exp_and_others set; the
    # table-load pass picks the first set containing each function, which
    # splits them across two sets and reloads tables on every Ln<->Exp
    # switch (~1.3us each). Restrict the map so the combined set wins.
    from concourse.hw_specs import get_activation_tables

    try:
        tabs = get_activation_tables(nc.m.arch)
    except Exception:
        return
    if "natural_log_exp_and_others" not in tabs:
        return
    for name, fns in tabs.items():
        if name != "natural_log_exp_and_others":
            fns.discard(Exp)
            fns.discard(Ln)


def _build():
    nc = bacc.Bacc(
        "TRN2", target_bir_lowering=False, debug=False, num_devices=NCORES
    )
    _patch_act_tables(nc)

    xT_ap = nc.dram_tensor("xT", [DPAD, BN], BF16, kind="ExternalInput").ap()
    wqT_ap = nc.dram_tensor("wqT", [DPAD, DHS], BF16, kind="ExternalInput").ap()
    wkT_ap = nc.dram_tensor("wkT", [DPAD, DHS], BF16, kind="ExternalInput").ap()
    wvT_ap = nc.dram_tensor("wvT", [DPAD, DHS], BF16, kind="ExternalInput").ap()
    woT_ap = nc.dram_tensor("woT", [DPAD, D - 1], BF16, kind="ExternalInput").ap()
    y_ap = nc.dram_tensor("y", [B * TPC, D], F32, kind="ExternalOutput").ap()

    with tile.TileContext(nc) as tc:
        with (
            tc.tile_pool(name="const", bufs=1) as constp,
            tc.tile_pool(name="w", bufs=1) as wp,
            tc.tile_pool(name="xT", bufs=10) as xtp,
            tc.tile_pool(name="qk", bufs=2) as qkp,
            tc.tile_pool(name="sq", bufs=2) as sqp,
            tc.tile_pool(name="va", bufs=2) as vap,
            tc.tile_pool(name="pt", bufs=3) as ptp,
            tc.tile_pool(name="sm", bufs=2) as smp,
            tc.tile_pool(name="d2", bufs=2) as d2p,
            tc.tile_pool(name="ps", bufs=2, space="PSUM") as psp,
            tc.tile_pool(name="acc", bufs=1, space="PSUM") as accp,
            tc.tile_pool(name="aux", bufs=2, space="PSUM") as auxp,
            tc.tile_pool(name="dram", bufs=1, space="DRAM") as dramp,
        ):
            ident = constp.tile([128, 128], F32)
            make_identity(nc, ident[:])
            ones65 = constp.tile([65, 1], F32)
            nc.vector.memset(ones65[:], 1.0)
            # block-diagonal ones: col 0 sums partitions 0-63 (q squares),
            # col 32 sums partitions 64-127 (k squares). 33 wide so the two
            # result rows land on engine-addressable partitions 0 and 32.
            bdiag = constp.tile([128, 33], BF16)
            nc.vector.memset(bdiag[:], 0.0)
            nc.vector.memset(bdiag[0:64, 0:1], 1.0)
            nc.vector.memset(bdiag[64:128, 32:33], 1.0)
            one1 = constp.tile([1, 1], F32)
            nc.vector.memset(one1[:], 1.0)
            bias10 = constp.tile([128, 1], F32)
            nc.vector.memset(bias10[:], INVK)
            biasD = constp.tile([128, 1], F32)
            nc.vector.memset(biasD[:], INVK * (1.0 + H * S_CONST * S_CONST))

            # Weights: [DPAD, S] viewed as [128, KC, S], bf16 from host
            wqb = wp.tile([128, KC, DHS], BF16)
            wkb = wp.tile([128, KC, DHS], BF16)
            wvb = wp.tile([128, KC, DHS], BF16)
            wob = wp.tile([128, KC, D - 1], BF16)
            for w_t, w_src in (
                (wqb, wqT_ap), (wkb, wkT_ap), (wvb, wvT_ap), (wob, woT_ap)
            ):
                nc.sync.dma_start(
                    w_t[:], w_src.rearrange("(k p) s -> p k s", p=128)
                )

            sends, recvs = [], []
            for b in range(B):
                sends.append(dramp.tile([N, DHS + 1], F32, tag=f"send{b}",
                                        name=f"send{b}"))
                recvs.append(dramp.tile([N, DHS + 1], F32, tag=f"recv{b}",
                                        name=f"recv{b}"))

            for b in range(B):
                # ============ Phase 1: per-batch attention (head h=core) ====
                # ---- load xT_b chunks [128, N] x 5 (bf16, host-cast)
                xtb = []
                for ki in range(KC):
                    t = xtp.tile([128, N], BF16, tag="xT", name=f"xb{b}_{ki}")
                    nc.sync.dma_start(
                        t[:],
                        xT_ap[ki * 128 : (ki + 1) * 128, b * N : (b + 1) * N],
                    )
                    xtb.append(t)

                # ---- v projection, natural layout [128, mi, 65] (col0 = t)
                va = vap.tile([128, N // 128, DHS + 1], BF16, tag="va")
                for mi in range(N // 128):
                    psv = auxp.tile([128, 512], F32, tag="aux",
                                    name=f"psv{b}_{mi}")
                    for ki in range(KC):
                        nc.tensor.matmul(
                            psv[:, 0:DHS],
                            xtb[ki][:, mi * 128 : (mi + 1) * 128],
                            wvb[:, ki, :],
                            start=(ki == 0),
                            stop=(ki == KC - 1),
                        )
                    nc.vector.tensor_copy(va[:, mi, 1:65], psv[:, 0:DHS])
                # batched: square all spatial entries, reduce per chunk,
                # then one Ln + one Exp for all 16 chunks
                vsq = smp.tile([128, N // 128, DHS], F32, tag="vsq")
                nc.vector.tensor_mul(vsq[:], va[:, :, 1:65], va[:, :, 1:65])
                vts = smp.tile([128, N // 128, 1], F32, tag="vts")
                nc.vector.reduce_sum(vts[:], vsq[:], axis=mybir.AxisListType.X)
                lnv = smp.tile([128, N // 128, 1], F32, tag="lnv")
                nc.scalar.activation(lnv[:], vts[:], Ln, bias=bias10[:])
                nc.scalar.activation(va[:, :, 0:1], lnv[:], Exp, scale=0.5)

                # ---- q/k projections -> [65, N] augmented (row 64 = +-t)
                qa = qkp.tile([65, N], BF16, tag="qa")
                ka = qkp.tile([65, N], BF16, tag="ka")
                for w_t, dst in ((wqb, qa), (wkb, ka)):
                    for nj in range(N // 512):
                        ps = auxp.tile([128, 512], F32, tag="aux",
                                       name=f"pqk{b}_{nj}")
                        for ki in range(KC):
                            nc.tensor.matmul(
                                ps[0:64, :],
                                w_t[:, ki, :],
                                xtb[ki][:, nj * 512 : (nj + 1) * 512],
                                start=(ki == 0),
                                stop=(ki == KC - 1),
                            )
                        nc.vector.tensor_copy(
                            dst[0:64, nj * 512 : (nj + 1) * 512], ps[0:64, :]
                        )
                # t rows for q AND k together: stack squares [128, N]
                # (rows 0-63 = q, 64-127 = k), block-diag ones-matmul
                # -> [2, 512] sums, Ln from PSUM, one Exp over [2, N]
                sqk = sqp.tile([128, N], BF16, tag="sqk")
                nc.vector.tensor_mul(sqk[0:64, :], qa[0:64, :], qa[0:64, :])
                nc.vector.tensor_mul(sqk[64:128, :], ka[0:64, :], ka[0:64, :])
                lrow = smp.tile([33, N], F32, tag="lrow")
                for nj in range(N // 512):
                    pst = auxp.tile([128, 512], F32, tag="aux",
                                    name=f"pst{b}_{nj}")
                    nc.tensor.matmul(
                        pst[0:33, :],
                        bdiag[:],
                        sqk[:, nj * 512 : (nj + 1) * 512],
                        start=True,
                        stop=True,
                    )
                    nc.scalar.activation(
                        lrow[:, nj * 512 : (nj + 1) * 512], pst[0:33, :], Ln,
                        bias=bias10[0:33, :],
                    )
                trow = smp.tile([33, N], BF16, tag="trow")
                nc.scalar.activation(trow[:], lrow[:], Exp, scale=0.5)
                nc.vector.tensor_copy(qa[64:65, :], trow[0:1, :])
                # k gets -t so the scores matmul computes the Lorentz
                # product q.k - t_q*t_k in one pass
                nc.vector.tensor_scalar_mul(ka[64:65, :], trow[32:33, :], -1.0)

                # ---- attention: scores^T -> exp -> m^T accumulation,
                # query-half at a time so exp runs on [128,1024] tiles
                mT = sqp.tile([65, N], F32, tag="mt")
                for h in range(2):
                    q0 = h * 1024
                    macc = [
                        accp.tile([65, 512], F32, tag=f"acc{j}",
                                  name=f"macc{b}_{h}_{j}")
                        for j in range(2)
                    ]
                    for mi in range(N // 128):
                        pss = psp.tile([128, 1024], F32, tag="ps")
                        for j in range(2):
                            nc.tensor.matmul(
                                pss[:, j * 512 : (j + 1) * 512],
                                ka[:, mi * 128 : (mi + 1) * 128],
                                qa[:, q0 + j * 512 : q0 + (j + 1) * 512],
                                start=True,
                                stop=True,
                            )
                        pt = ptp.tile([128, 1024], BF16, tag="pt")
                        nc.scalar.activation(pt[:], pss[:], Exp, scale=SCALE)
                        for j in range(2):
                            nc.tensor.matmul(
                                macc[j][:],
                                va[:, mi, :],
                                pt[:, j * 512 : (j + 1) * 512],
                                start=(mi == 0),
                                stop=(mi == N // 128 - 1),
                            )
                    for j in range(2):
                        nc.vector.tensor_copy(
                            mT[:, q0 + j * 512 : q0 + (j + 1) * 512],
                            macc[j][:],
                        )

                # ---- Lorentz midpoint normalize (transposed layout)
                sqT = sqp.tile([65, N], F32, tag="sq")
                nc.vector.tensor_mul(sqT[:], mT[:], mT[:])
                rT = smp.tile([1, N], F32, tag="row")
                for nj in range(N // 512):
                    psc = auxp.tile([128, 512], F32, tag="aux",
                                    name=f"psc{b}_{nj}")
                    nc.tensor.matmul(
                        psc[0:1, :],
                        ones65[:],
                        sqT[:, nj * 512 : (nj + 1) * 512],
                        start=True,
                        stop=True,
                    )
                    # r = 2*t^2 - sum_all(sq)  (= t^2 - ||space||^2)
                    t2c = smp.tile([1, 512], F32, tag="t2")
                    nc.vector.tensor_scalar_mul(
                        t2c[:], sqT[0:1, nj * 512 : (nj + 1) * 512], 2.0
                    )
                    nc.vector.tensor_sub(
                        rT[:, nj * 512 : (nj + 1) * 512], t2c[:], psc[0:1, :]
                    )
                # rotate r into token-partition layout via K=1 matmuls,
                # then one Ln + one Exp for all 16 chunks
                prl = auxp.tile([128, 512], F32, tag="aux", name=f"prl{b}")
                for j in range(N // 128):
                    nc.tensor.matmul(
                        prl[:, j : j + 1],
                        rT[:, j * 128 : (j + 1) * 128],
                        one1[:],
                        start=True,
                        stop=True,
                    )
                lnr = smp.tile([128, N // 128], F32, tag="lnr")
                nc.scalar.activation(lnr[:], prl[:, 0 : N // 128], Ln,
                                     scale=KCURV)
                rinv = smp.tile([128, N // 128], F32, tag="rinv")
                nc.scalar.activation(rinv[:], lnr[:], Exp, scale=-0.5)
                for nj2 in range(N // 128):
                    ptr2 = auxp.tile([128, 512], F32, tag="aux",
                                     name=f"ptr{b}_{nj2}")
                    nc.tensor.transpose(
                        ptr2[0:128, 0:65], mT[:, nj2 * 128 : (nj2 + 1) * 128],
                        ident[0:65, 0:65],
                    )
                    mo = smp.tile([128, DHS + 1], F32, tag="mo", bufs=4)
                    nc.vector.tensor_scalar_mul(
                        mo[:], ptr2[0:128, 0:65], rinv[:, nj2 : nj2 + 1]
                    )
                    nc.sync.dma_start(
                        sends[b][nj2 * 128 : (nj2 + 1) * 128, :], mo[:]
                    )

                # ============ Phase 2 (per batch): exchange + fusion ========
                nc.gpsimd.collective_compute(
                    "AllToAll",
                    mybir.AluOpType.bypass,
                    replica_groups=[list(range(NCORES))],
                    ins=[sends[b].opt()],
                    outs=[recvs[b].opt()],
                )
                # recv rows: j*256 + q*128 + p  (j = head, q = row chunk)
                recv_r = recvs[b][:].rearrange(
                    "(j q p) d -> q p j d", j=H, q=TPC // 128, p=128
                )

                rvs = []
                tsA = smp.tile([128, TPC // 128], F32, tag="tsA")
                for r in range(TPC // 128):
                    rv = d2p.tile([128, H, DHS + 1], F32, tag="rv", bufs=4,
                                  name=f"rv{b}_{r}")
                    nc.sync.dma_start(rv[:], recv_r[r])
                    rvs.append(rv)
                    tsq = smp.tile([128, H, 1], F32, tag="tsq")
                    nc.vector.tensor_mul(tsq[:], rv[:, :, 0:1], rv[:, :, 0:1])
                    nc.vector.reduce_sum(
                        tsA[:, r : r + 1], tsq[:, :, 0],
                        axis=mybir.AxisListType.X,
                    )
                # t' = exp(.5 ln(s^2 * sum_h t_h^2 + INVK*(1+H*s^2)))
                lnt2 = smp.tile([128, TPC // 128], F32, tag="lnt2")
                nc.scalar.activation(
                    lnt2[:], tsA[:], Ln, scale=S_CONST * S_CONST, bias=biasD[:]
                )
                tpA = smp.tile([128, TPC // 128], F32, tag="tpA")
                nc.scalar.activation(tpA[:], lnt2[:], Exp, scale=0.5)

                outts = []
                osA = smp.tile([128, TPC // 128], F32, tag="osA")
                for r in range(TPC // 128):
                    rv = rvs[r]
                    fu = d2p.tile([128, DPAD], F32, tag="fu", bufs=2)
                    nc.vector.tensor_copy(fu[:, 0:1], tpA[:, r : r + 1])
                    nc.vector.tensor_scalar_mul(
                        fu[:, 1:513].rearrange("p (j s) -> p j s", j=H),
                        rv[:, :, 1:65],
                        S_CONST,
                    )
                    nc.vector.memset(fu[:, 513:514], 1.0)
                    nc.vector.memset(fu[:, 514:DPAD], 0.0)

                    # transpose to [d, tokens] for the output contraction
                    ft = d2p.tile([128, KC, 128], BF16, tag="ft")
                    for ki in range(KC):
                        pstr = auxp.tile([128, 512], F32, tag="aux",
                                         name=f"pf{b}_{r}_{ki}")
                        nc.tensor.transpose(
                            pstr[:, 0:128],
                            fu[:, ki * 128 : (ki + 1) * 128], ident[:]
                        )
                        nc.vector.tensor_copy(ft[:, ki, :], pstr[:, 0:128])

                    # output projection [128 tokens, 512]
                    pso = auxp.tile([128, 512], F32, tag="aux",
                                    name=f"po{b}_{r}")
                    for ki in range(KC):
                        nc.tensor.matmul(
                            pso[:],
                            ft[:, ki, :],
                            wob[:, ki, :],
                            start=(ki == 0),
                            stop=(ki == KC - 1),
                        )
                    outt = d2p.tile([128, D], F32, tag="out", bufs=4,
                                    name=f"outt{b}_{r}")
                    nc.vector.tensor_copy(outt[:, 1:D], pso[:])
                    outts.append(outt)
                    osq = smp.tile([128, 512], F32, tag="osq")
                    nc.vector.tensor_mul(osq[:], outt[:, 1:D], outt[:, 1:D])
                    nc.vector.reduce_sum(
                        osA[:, r : r + 1], osq[:], axis=mybir.AxisListType.X
                    )
                # batched t_out = exp(.5 ln(sum + INVK)), then store
                lno = smp.tile([128, TPC // 128], F32, tag="lno")
                nc.scalar.activation(lno[:], osA[:], Ln, bias=bias10[:])
                toA = smp.tile([128, TPC // 128], F32, tag="toA")
                nc.scalar.activation(toA[:], lno[:], Exp, scale=0.5)
                for r in range(TPC // 128):
                    nc.vector.tensor_copy(
                        outts[r][:, 0:1], toA[:, r : r + 1]
                    )
                    nc.sync.dma_start(
                        y_ap[b * TPC + r * 128 : b * TPC + (r + 1) * 128, :],
                        outts[r][:],
                    )

    nc.compile()
    return nc


def _to_bf16(a):
    return np.asarray(a, dtype=np.float32).astype(ml_dtypes.bfloat16)


def _prep_inputs(x, Wq, bq, Wk, bk, Wv, bv, Wo, bo):
    xT = np.zeros((DPAD, BN), dtype=np.float32)
    xT[:D, :] = np.ascontiguousarray(x.reshape(BN, D).T)
    xT[D, :] = 1.0
    xT = _to_bf16(xT)

    woT = np.zeros((DPAD, D - 1), dtype=np.float32)
    woT[:D + 1, :] = np.concatenate([Wo.T, bo[None, :]], axis=0)
    woT = _to_bf16(woT)

    in_maps = []
    for h in range(NCORES):
        m = {"xT": xT, "woT": woT}
        for nm, W, bvec in (("wqT", Wq, bq), ("wkT", Wk, bk), ("wvT", Wv, bv)):
            w = np.zeros((DPAD, DHS), dtype=np.float32)
            w[0:D + 1, :] = np.concatenate([W[h].T, bvec[h][None, :]], axis=0)
            m[nm] = _to_bf16(w)
        in_maps.append(m)
    return in_maps


def _run(inputs, trace=False, **kw):
    if "nc" not in _CACHE:
        _CACHE["nc"] = _build()
    nc = _CACHE["nc"]
    in_maps = _prep_inputs(**{k: np.asarray(v) for k, v in inputs.items()})
    res = bass_utils.run_bass_kernel_spmd(
        nc, in_maps, core_ids=list(range(NCORES)), trace=trace, **kw
    )
    # core c's y holds, for each batch b, tokens [c*TPC, (c+1)*TPC)
    y = np.zeros((B, N, D), dtype=np.float32)
    for c in range(NCORES):
        yc = res.results[c]["y"].reshape(B, TPC, D)
        y[:, c * TPC : (c + 1) * TPC, :] = yc
    return y, res


def kernel(**inputs):
    y, _ = _run(inputs)
    return y


# revision 5
# speedup vs baseline: 1.3013x; 1.3013x over previous
"""Lorentz multi-head attention on 8 Trainium2 NeuronCores.

Sharding: head-parallel phase 1 (core c computes head c for all batches:
QKV Lorentz projections, Lorentz-inner-product scores, softmax-free
exp-attention, Lorentz-midpoint normalize), then a PER-BATCH AllToAll
exchanges (head-block -> token-block) so phase 2 (concat_logradius fusion
+ output LorentzFC) runs token-parallel (core c handles tokens
[c*256,(c+1)*256) of each batch).

Pipelining: batch b+1's projection matmuls are interleaved into batch b's
(ScalarE-bound) attention loop so the PE never idles long enough for the
HAM clock-gate to re-throttle it to 1.2 GHz; phase 2 of batch b-1 is
emitted after phase 1 of batch b so its AllToAll-wait is already
satisfied when the FIFO engine queues reach it.

Softmax denominator is skipped entirely: the Lorentz midpoint renormalizes
m / sqrt(K*(t^2-||s||^2)), which is invariant to positive row scaling, so
exp(scores) can be used unnormalized (scores are O(+-5), no overflow risk).

Biases are folded into the matmuls by augmenting tokens with a constant-1
column and weights with a bias row. sqrt/rsqrt are computed as
exp(+-0.5*ln(x)) so the ScalarEngine needs only the one
natural_log_exp_and_others table set (no ~2.7us table swaps).

Inputs arrive pre-transposed AND pre-cast to bf16 on the host. Score
exp() runs on [128,1024] two-bank PSUM tiles; q and k time-rows come from
one block-diagonal ones-matmul + Ln-from-PSUM + a single [33,2048] Exp.
"""

import sys

sys.path.insert(0, "/opt/trn_rl_repo")

import numpy as np
import ml_dtypes

import concourse.bass as bass
import concourse.mybir as mybir
import concourse.tile as tile
from concourse import bacc, bass_utils
from concourse.masks import make_identity

# Problem constants (hardcoded per task contract)
B, N, D = 4, 2048, 513
H, DHS = 8, 64
NCORES = 8
KCURV = 0.1
INVK = 10.0
SCALE = 1.0 / np.sqrt(DHS)  # 0.125
S_CONST = 2.8479428291320801  # exp(0.5*(digamma(256)-digamma(32)))
DPAD = 640  # 513 padded to 5*128 (col 513 = constant-1 bias lane)
KC = 5  # contraction chunks of 128
BN = B * N  # 8192 tokens
TPC = N // NCORES  # 256 tokens per core per batch in phase 2
F32 = mybir.dt.float32
BF16 = mybir.dt.bfloat16
Ln = mybir.ActivationFunctionType.Ln
Exp = mybir.ActivationFunctionType.Exp

_CACHE = {}


def _patch_act_tables(nc):
    # Exp and Ln both live in the natural_log_exp_and_others set; the
    # table-load pass picks the first set containing each function, which
    # splits them across two sets and reloads tables on every Ln<->Exp
    # switch (~1.3us each). Restrict the map so the combined set wins.
    from concourse.hw_specs import get_activation_tables

    try:
        tabs = get_activation_tables(nc.m.arch)
    except Exception:
        return
    if "natural_log_exp_and_others" not in tabs:
        return
    for name, fns in tabs.items():
        if name != "natural_log_exp_and_others":
            fns.discard(Exp)
            fns.discard(Ln)


def _build():
    nc = bacc.Bacc(
        "TRN2", target_bir_lowering=False, debug=False, num_devices=NCORES
    )
    _patch_act_tables(nc)

    xT_ap = nc.dram_tensor("xT", [DPAD, BN], BF16, kind="ExternalInput").ap()
    wqT_ap = nc.dram_tensor("wqT", [DPAD, DHS], BF16, kind="ExternalInput").ap()
    wkT_ap = nc.dram_tensor("wkT", [DPAD, DHS], BF16, kind="ExternalInput").ap()
    wvT_ap = nc.dram_tensor("wvT", [DPAD, DHS], BF16, kind="ExternalInput").ap()
    woT_ap = nc.dram_tensor("woT", [DPAD, D - 1], BF16, kind="ExternalInput").ap()
    y_ap = nc.dram_tensor("y", [B * TPC, D], F32, kind="ExternalOutput").ap()

    with tile.TileContext(nc) as tc:
        with (
            tc.tile_pool(name="const", bufs=1) as constp,
            tc.tile_pool(name="w", bufs=1) as wp,
            tc.tile_pool(name="xT", bufs=10) as xtp,
            tc.tile_pool(name="qk", bufs=2) as qkp,
            tc.tile_pool(name="sq", bufs=2) as sqp,
            tc.tile_pool(name="va", bufs=2) as vap,
            tc.tile_pool(name="pt", bufs=3) as ptp,
            tc.tile_pool(name="sm", bufs=2) as smp,
            tc.tile_pool(name="d2", bufs=2) as d2p,
            tc.tile_pool(name="ps", bufs=2, space="PSUM") as psp,
            tc.tile_pool(name="acc", bufs=1, space="PSUM") as accp,
            tc.tile_pool(name="aux", bufs=2, space="PSUM") as auxp,
            tc.tile_pool(name="dram", bufs=1, space="DRAM") as dramp,
        ):
            ident = constp.tile([128, 128], F32)
            make_identity(nc, ident[:])
            ones65 = constp.tile([65, 1], F32)
            nc.vector.memset(ones65[:], 1.0)
            # block-diagonal ones: col 0 sums partitions 0-63 (q squares),
            # col 32 sums partitions 64-127 (k squares). 33 wide so the two
            # result rows land on engine-addressable partitions 0 and 32.
            bdiag = constp.tile([128, 33], BF16)
            nc.vector.memset(bdiag[:], 0.0)
            nc.vector.memset(bdiag[0:64, 0:1], 1.0)
            nc.vector.memset(bdiag[64:128, 32:33], 1.0)
            one1 = constp.tile([1, 1], F32)
            nc.vector.memset(one1[:], 1.0)
            bias10 = constp.tile([128, 1], F32)
            nc.vector.memset(bias10[:], INVK)
            biasD = constp.tile([128, 1], F32)
            nc.vector.memset(biasD[:], INVK * (1.0 + H * S_CONST * S_CONST))

            # Weights: [DPAD, S] viewed as [128, KC, S], bf16 from host
            wqb = wp.tile([128, KC, DHS], BF16)
            wkb = wp.tile([128, KC, DHS], BF16)
            wvb = wp.tile([128, KC, DHS], BF16)
            wob = wp.tile([128, KC, D - 1], BF16)
            for w_t, w_src in (
                (wqb, wqT_ap), (wkb, wkT_ap), (wvb, wvT_ap), (wob, woT_ap)
            ):
                nc.sync.dma_start(
                    w_t[:], w_src.rearrange("(k p) s -> p k s", p=128)
                )

            sends, recvs = [], []
            for b in range(B):
                sends.append(dramp.tile([N, DHS + 1], F32, tag=f"send{b}",
                                        name=f"send{b}"))
                recvs.append(dramp.tile([N, DHS + 1], F32, tag=f"recv{b}",
                                        name=f"recv{b}"))

            def load_xtb(b):
                xtb = []
                for ki in range(KC):
                    t = xtp.tile([128, N], BF16, tag="xT", name=f"xb{b}_{ki}")
                    nc.sync.dma_start(
                        t[:],
                        xT_ap[ki * 128 : (ki + 1) * 128, b * N : (b + 1) * N],
                    )
                    xtb.append(t)
                return xtb

            def proj_units(b, xtb):
                """Emission units for batch b's q/k/v projections; called
                one-at-a-time interleaved into batch b-1's attention loop.
                Returns (units, handles-dict)."""
                va = vap.tile([128, N // 128, DHS + 1], BF16, tag="va",
                              name=f"va{b}")
                qa = qkp.tile([65, N], BF16, tag="qa", name=f"qa{b}")
                ka = qkp.tile([65, N], BF16, tag="ka", name=f"ka{b}")
                units = []

                # v projection: [128, mi, 65] natural layout (col0 = t)
                def v_unit(mi):
                    psv = auxp.tile([128, 512], F32, tag="aux",
                                    name=f"psv{b}_{mi}")
                    for ki in range(KC):
                        nc.tensor.matmul(
                            psv[:, 0:DHS],
                            xtb[ki][:, mi * 128 : (mi + 1) * 128],
                            wvb[:, ki, :],
                            start=(ki == 0),
                            stop=(ki == KC - 1),
                        )
                    nc.vector.tensor_copy(va[:, mi, 1:65], psv[:, 0:DHS])

                def v_finish():
                    vsq = smp.tile([128, N // 128, DHS], F32, tag="vsq")
                    nc.vector.tensor_mul(vsq[:], va[:, :, 1:65],
                                         va[:, :, 1:65])
                    vts = smp.tile([128, N // 128, 1], F32, tag="vts")
                    nc.vector.reduce_sum(vts[:], vsq[:],
                                         axis=mybir.AxisListType.X)
                    lnv = smp.tile([128, N // 128, 1], F32, tag="lnv")
                    nc.scalar.activation(lnv[:], vts[:], Ln, bias=bias10[:])
                    nc.scalar.activation(va[:, :, 0:1], lnv[:], Exp, scale=0.5)

                # q/k spatial projections -> rows 0..63 of [65, N]
                def qk_unit(w_t, dst, nj):
                    ps = auxp.tile([128, 512], F32, tag="aux",
                                   name=f"pqk{b}_{nj}")
                    for ki in range(KC):
                        nc.tensor.matmul(
                            ps[0:64, :],
                            w_t[:, ki, :],
                            xtb[ki][:, nj * 512 : (nj + 1) * 512],
                            start=(ki == 0),
                            stop=(ki == KC - 1),
                        )
                    nc.vector.tensor_copy(
                        dst[0:64, nj * 512 : (nj + 1) * 512], ps[0:64, :]
                    )

                # t rows for q AND k: stacked squares [128, N], block-diag
                # ones-matmul -> [33,512] sums (rows 0 / 32), Ln from PSUM,
                # one [33, N] Exp, then copy/negate into row 64
                sqk = sqp.tile([128, N], BF16, tag="sqk", name=f"sqk{b}")
                lrow = smp.tile([33, N], F32, tag="lrow", name=f"lrow{b}")

                def sq_unit(which):
                    if which == 0:
                        nc.vector.tensor_mul(sqk[0:64, :], qa[0:64, :],
                                             qa[0:64, :])
                    else:
                        nc.vector.tensor_mul(sqk[64:128, :], ka[0:64, :],
                                             ka[0:64, :])

                def srow_unit(nj):
                    pst = auxp.tile([128, 512], F32, tag="aux",
                                    name=f"pst{b}_{nj}")
                    nc.tensor.matmul(
                        pst[0:33, :],
                        bdiag[:],
                        sqk[:, nj * 512 : (nj + 1) * 512],
                        start=True,
                        stop=True,
                    )
                    nc.scalar.activation(
                        lrow[:, nj * 512 : (nj + 1) * 512], pst[0:33, :], Ln,
                        bias=bias10[0:33, :],
                    )

                def t_finish():
                    trow = smp.tile([33, N], BF16, tag="trow",
                                    name=f"trow{b}")
                    nc.scalar.activation(trow[:], lrow[:], Exp, scale=0.5)
                    nc.vector.tensor_copy(qa[64:65, :], trow[0:1, :])
                    # k gets -t so the scores matmul computes the Lorentz
                    # product q.k - t_q*t_k in one pass
                    nc.vector.tensor_scalar_mul(ka[64:65, :], trow[32:33, :],
                                                -1.0)

                for mi in range(N // 128):
                    units.append(lambda mi=mi: v_unit(mi))
                units.append(v_finish)
                for nj in range(N // 512):
                    units.append(lambda nj=nj: qk_unit(wqb, qa, nj))
                for nj in range(N // 512):
                    units.append(lambda nj=nj: qk_unit(wkb, ka, nj))
                units.append(lambda: sq_unit(0))
                units.append(lambda: sq_unit(1))
                for nj in range(N // 512):
                    units.append(lambda nj=nj: srow_unit(nj))
                units.append(t_finish)
                return units, {"va": va, "qa": qa, "ka": ka}

            def attention(b, hd, interleave):
                """Scores -> exp -> midpoint accumulation for batch b,
                popping one interleave unit per mi iteration."""
                qa, ka, va = hd["qa"], hd["ka"], hd["va"]
                ui = 0
                mT = sqp.tile([65, N], F32, tag="mt", name=f"mT{b}")
                for h in range(2):
                    q0 = h * 1024
                    macc = [
                        accp.tile([65, 512], F32, tag=f"acc{j}",
                                  name=f"macc{b}_{h}_{j}")
                        for j in range(2)
                    ]
                    for mi in range(N // 128):
                        pss = psp.tile([128, 1024], F32, tag="ps")
                        for j in range(2):
                            nc.tensor.matmul(
                                pss[:, j * 512 : (j + 1) * 512],
                                ka[:, mi * 128 : (mi + 1) * 128],
                                qa[:, q0 + j * 512 : q0 + (j + 1) * 512],
                                start=True,
                                stop=True,
                            )
                        pt = ptp.tile([128, 1024], BF16, tag="pt")
                        nc.scalar.activation(pt[:], pss[:], Exp, scale=SCALE)
                        for j in range(2):
                            nc.tensor.matmul(
                                macc[j][:],
                                va[:, mi, :],
                                pt[:, j * 512 : (j + 1) * 512],
                                start=(mi == 0),
                                stop=(mi == N // 128 - 1),
                            )
                        if ui < len(interleave):
                            interleave[ui]()
                            ui += 1
                    for j in range(2):
                        nc.vector.tensor_copy(
                            mT[:, q0 + j * 512 : q0 + (j + 1) * 512],
                            macc[j][:],
                        )
                while ui < len(interleave):
                    interleave[ui]()
                    ui += 1
                return mT

            def midpoint_and_send(b, mT):
                """Lorentz midpoint normalize (transposed layout), store
                to the per-batch A2A send buffer, trigger the AllToAll."""
                sqT = sqp.tile([65, N], F32, tag="sq", name=f"sqT{b}")
                nc.vector.tensor_mul(sqT[:], mT[:], mT[:])
                rT = smp.tile([1, N], F32, tag="row", name=f"rT{b}")
                for nj in range(N // 512):
                    psc = auxp.tile([128, 512], F32, tag="aux",
                                    name=f"psc{b}_{nj}")
                    nc.tensor.matmul(
                        psc[0:1, :],
                        ones65[:],
                        sqT[:, nj * 512 : (nj + 1) * 512],
                        start=True,
                        stop=True,
                    )
                    # r = 2*t^2 - sum_all(sq)  (= t^2 - ||space||^2)
                    t2c = smp.tile([1, 512], F32, tag="t2")
                    nc.vector.tensor_scalar_mul(
                        t2c[:], sqT[0:1, nj * 512 : (nj + 1) * 512], 2.0
                    )
                    nc.vector.tensor_sub(
                        rT[:, nj * 512 : (nj + 1) * 512], t2c[:], psc[0:1, :]
                    )
                # rotate r into token-partition layout via K=1 matmuls,
                # then one Ln + one Exp for all 16 chunks
                prl = auxp.tile([128, 512], F32, tag="aux", name=f"prl{b}")
                for j in range(N // 128):
                    nc.tensor.matmul(
                        prl[:, j : j + 1],
                        rT[:, j * 128 : (j + 1) * 128],
                        one1[:],
                        start=True,
                        stop=True,
                    )
                lnr = smp.tile([128, N // 128], F32, tag="lnr")
                nc.scalar.activation(lnr[:], prl[:, 0 : N // 128], Ln,
                                     scale=KCURV)
                rinv = smp.tile([128, N // 128], F32, tag="rinv")
                nc.scalar.activation(rinv[:], lnr[:], Exp, scale=-0.5)
                for nj2 in range(N // 128):
                    ptr2 = auxp.tile([128, 512], F32, tag="aux",
                                     name=f"ptr{b}_{nj2}")
                    nc.tensor.transpose(
                        ptr2[0:128, 0:65], mT[:, nj2 * 128 : (nj2 + 1) * 128],
                        ident[0:65, 0:65],
                    )
                    mo = smp.tile([128, DHS + 1], F32, tag="mo", bufs=4)
                    nc.vector.tensor_scalar_mul(
                        mo[:], ptr2[0:128, 0:65], rinv[:, nj2 : nj2 + 1]
                    )
                    nc.sync.dma_start(
                        sends[b][nj2 * 128 : (nj2 + 1) * 128, :], mo[:]
                    )
                nc.gpsimd.collective_compute(
                    "AllToAll",
                    mybir.AluOpType.bypass,
                    replica_groups=[list(range(NCORES))],
                    ins=[sends[b].opt()],
                    outs=[recvs[b].opt()],
                )

            def phase2(b):
                """Per-batch concat_logradius fusion + output LorentzFC on
                this core's 256 tokens; emitted one batch late so the
                AllToAll-wait is already satisfied."""
                recv_r = recvs[b][:].rearrange(
                    "(j q p) d -> q p j d", j=H, q=TPC // 128, p=128
                )
                rvs = []
                tsA = smp.tile([128, TPC // 128], F32, tag="tsA")
                for r in range(TPC // 128):
                    rv = d2p.tile([128, H, DHS + 1], F32, tag="rv", bufs=4,
                                  name=f"rv{b}_{r}")
                    nc.sync.dma_start(rv[:], recv_r[r])
                    rvs.append(rv)
                    tsq = smp.tile([128, H, 1], F32, tag="tsq")
                    nc.vector.tensor_mul(tsq[:], rv[:, :, 0:1], rv[:, :, 0:1])
                    nc.vector.reduce_sum(
                        tsA[:, r : r + 1], tsq[:, :, 0],
                        axis=mybir.AxisListType.X,
                    )
                # t' = exp(.5 ln(s^2 * sum_h t_h^2 + INVK*(1+H*s^2)))
                lnt2 = smp.tile([128, TPC // 128], F32, tag="lnt2")
                nc.scalar.activation(
                    lnt2[:], tsA[:], Ln, scale=S_CONST * S_CONST,
                    bias=biasD[:],
                )
                tpA = smp.tile([128, TPC // 128], F32, tag="tpA")
                nc.scalar.activation(tpA[:], lnt2[:], Exp, scale=0.5)

                outts = []
                osA = smp.tile([128, TPC // 128], F32, tag="osA")
                for r in range(TPC // 128):
                    rv = rvs[r]
                    fu = d2p.tile([128, DPAD], F32, tag="fu", bufs=2)
                    nc.vector.tensor_copy(fu[:, 0:1], tpA[:, r : r + 1])
                    nc.vector.tensor_scalar_mul(
                        fu[:, 1:513].rearrange("p (j s) -> p j s", j=H),
                        rv[:, :, 1:65],
                        S_CONST,
                    )
                    nc.vector.memset(fu[:, 513:514], 1.0)
                    nc.vector.memset(fu[:, 514:DPAD], 0.0)

                    # transpose to [d, tokens] for the output contraction
                    ft = d2p.tile([128, KC, 128], BF16, tag="ft")
                    for ki in range(KC):
                        pstr = auxp.tile([128, 512], F32, tag="aux",
                                         name=f"pf{b}_{r}_{ki}")
                        nc.tensor.transpose(
                            pstr[:, 0:128],
                            fu[:, ki * 128 : (ki + 1) * 128], ident[:]
                        )
                        nc.vector.tensor_copy(ft[:, ki, :], pstr[:, 0:128])

                    # output projection [128 tokens, 512]
                    pso = auxp.tile([128, 512], F32, tag="aux",
                                    name=f"po{b}_{r}")
                    for ki in range(KC):
                        nc.tensor.matmul(
                            pso[:],
                            ft[:, ki, :],
                            wob[:, ki, :],
                            start=(ki == 0),
                            stop=(ki == KC - 1),
                        )
                    outt = d2p.tile([128, D], F32, tag="out", bufs=4,
                                    name=f"outt{b}_{r}")
                    nc.vector.tensor_copy(outt[:, 1:D], pso[:])
                    outts.append(outt)
                    osq = smp.tile([128, 512], F32, tag="osq")
                    nc.vector.tensor_mul(osq[:], outt[:, 1:D], outt[:, 1:D])
                    nc.vector.reduce_sum(
                        osA[:, r : r + 1], osq[:], axis=mybir.AxisListType.X
                    )
                # batched t_out = exp(.5 ln(sum + INVK)), then store
                lno = smp.tile([128, TPC // 128], F32, tag="lno")
                nc.scalar.activation(lno[:], osA[:], Ln, bias=bias10[:])
                toA = smp.tile([128, TPC // 128], F32, tag="toA")
                nc.scalar.activation(toA[:], lno[:], Exp, scale=0.5)
                for r in range(TPC // 128):
                    nc.vector.tensor_copy(
                        outts[r][:, 0:1], toA[:, r : r + 1]
                    )
                    nc.sync.dma_start(
                        y_ap[b * TPC + r * 128 : b * TPC + (r + 1) * 128, :],
                        outts[r][:],
                    )

            # ---------------- pipelined emission schedule ----------------
            xtb = load_xtb(0)
            units, hd = proj_units(0, xtb)
            for u in units:
                u()
            for b in range(B):
                if b + 1 < B:
                    xtb_n = load_xtb(b + 1)
                    units_n, hd_n = proj_units(b + 1, xtb_n)
                else:
                    units_n, hd_n = [], None
                mT = attention(b, hd, units_n)
                midpoint_and_send(b, mT)
                if b >= 1:
                    phase2(b - 1)
                hd = hd_n
            phase2(B - 1)

    nc.compile()
    return nc


def _to_bf16(a):
    return np.asarray(a, dtype=np.float32).astype(ml_dtypes.bfloat16)


def _prep_inputs(x, Wq, bq, Wk, bk, Wv, bv, Wo, bo):
    xT = np.zeros((DPAD, BN), dtype=np.float32)
    xT[:D, :] = np.ascontiguousarray(x.reshape(BN, D).T)
    xT[D, :] = 1.0
    xT = _to_bf16(xT)

    woT = np.zeros((DPAD, D - 1), dtype=np.float32)
    woT[:D + 1, :] = np.concatenate([Wo.T, bo[None, :]], axis=0)
    woT = _to_bf16(woT)

    in_maps = []
    for h in range(NCORES):
        m = {"xT": xT, "woT": woT}
        for nm, W, bvec in (("wqT", Wq, bq), ("wkT", Wk, bk), ("wvT", Wv, bv)):
            w = np.zeros((DPAD, DHS), dtype=np.float32)
            w[0:D + 1, :] = np.concatenate([W[h].T, bvec[h][None, :]], axis=0)
            m[nm] = _to_bf16(w)
        in_maps.append(m)
    return in_maps


def _run(inputs, trace=False, **kw):
    if "nc" not in _CACHE:
        _CACHE["nc"] = _build()
    nc = _CACHE["nc"]
    in_maps = _prep_inputs(**{k: np.asarray(v) for k, v in inputs.items()})
    res = bass_utils.run_bass_kernel_spmd(
        nc, in_maps, core_ids=list(range(NCORES)), trace=trace, **kw
    )
    # core c's y holds, for each batch b, tokens [c*TPC, (c+1)*TPC)
    y = np.zeros((B, N, D), dtype=np.float32)
    for c in range(NCORES):
        yc = res.results[c]["y"].reshape(B, TPC, D)
        y[:, c * TPC : (c + 1) * TPC, :] = yc
    return y, res


def kernel(**inputs):
    y, _ = _run(inputs)
    return y
